# revision 14
# baseline (speedup 1.0000x reference)
"""Non-local (dot-product attention) block kernel for Trainium2, 8 cores.

Reference math (per sample):
    t = theta_w @ xf + theta_b           (D, N)
    p = (phi_w @ xf + phi_b) / N         (D, N)
    g = g_w @ xf + g_b                   (D, N)
    f = t.T p  (NxN attention);  y = f g.T;  z = BN(w_w y) + x

Algebraic collapse (matmul associativity, BN folded on host):
    M[e,d]  = sum_m p[e,m] g[d,m]                      (D x D)
    V[c,e]  = sum_d w'[c,d] M[e,d]      w' = diag(inv) w_w
    U[c,ci] = sum_e V[c,e] theta_w[e,ci]               (C x C)
    b_z[c]  = sum_e V[c,e] theta_b[e] + b'[c]
    z       = U @ xf + b_z 1^T + x
so the N x N attention matrix, y, and the theta projection never exist --
per sample just two passes over x (proj p|g, final U @ x) plus tiny
D x D / C x C matmuls in between.

Sharding: data-parallel over batch B=8, one sample per NeuronCore, no
collectives. Matmul inputs bf16 (fp32 PSUM accumulation), biases and
residual applied in fp32; output fp32.

HW notes baked in:
  - All weights/biases ship as ONE byte-packed DMA (each dma_start has a
    ~600ns fixed cost and small rows kill DMA descriptor throughput).
  - x halves are monolithic (128, 3072) bf16 DMAs (6KB descriptors) split
    across the two HWDGE rings (sync + scalar) for parallel issue.
  - Dummy matmuls bridge the small-matmul chain between the pg phase and
    the z phase so the PE HAM clock stays at 2.4 GHz for the z matmuls.
  - Output DMAs alternate rings, (128, 1024) fp32 chunks.
"""

import numpy as np

B, C, HH, WW = 8, 256, 96, 32
N = HH * WW          # 3072
D = 128              # inter_channels
BN_EPS = 1e-5
NT = N // 128        # 24 pixel chunks
NR = N // 1024       # 3 pixel regions
N_CORES = 8

_NC = None


def _build_nc():
    from contextlib import ExitStack

    import concourse.bass as bass
    import concourse.bacc as bacc
    import concourse.tile as tile
    from concourse import mybir

    f32 = mybir.dt.float32
    bf16 = mybir.dt.bfloat16
    AF = mybir.ActivationFunctionType
    ALU = mybir.AluOpType

    nc = bacc.Bacc(
        "TRN2",
        target_bir_lowering=False,
        debug=False,
        num_devices=N_CORES,
    )

    x = nc.dram_tensor("x", [C, N], bf16, kind="ExternalInput").ap()
    # wpk byte-packs, per partition row: aux 260 f32 | pgW 512 bf16 |
    # thw 260 bf16 | wT 256 bf16  => 774 f32 columns total
    wpk = nc.dram_tensor("wpk", [128, 774], f32, kind="ExternalInput").ap()
    out = nc.dram_tensor("out", [C, N], f32, kind="ExternalOutput").ap()

    with tile.TileContext(nc) as tc, ExitStack() as ctx:
        const = ctx.enter_context(tc.tile_pool(name="const", bufs=1))
        zpool = ctx.enter_context(tc.tile_pool(name="zpool", bufs=3))
        ps_mm = ctx.enter_context(tc.tile_pool(name="ps_mm", bufs=3, space="PSUM"))
        ps_sm = ctx.enter_context(tc.tile_pool(name="ps_sm", bufs=1, space="PSUM"))

        X0 = const.tile([128, N], bf16)
        X1 = const.tile([128, N], bf16)
        pg_sb = const.tile([128, NT * 256], bf16)
        m2_sb = const.tile([128, 128], bf16)
        w2_sb = const.tile([128, 256], bf16)
        ut_sb = const.tile([128, 512], bf16)
        bz_sb = const.tile([128, 2], f32)
        wpk_sb = const.tile([128, 774], f32)

        # in-DMA bandwidth is one shared ~320GB/s pool, so order strictly by
        # need: packed weights (warm-up fodder), then X0 (k=0 matmuls), X1
        nc.sync.dma_start(out=wpk_sb, in_=wpk)
        nc.sync.dma_start(out=X0, in_=x[0:128, :])
        nc.sync.dma_start(out=X1, in_=x[128:256, :])

        aux_sb = wpk_sb[:, 0:260]
        pgW = wpk_sb[:, 260:516].bitcast(bf16)     # (128, 512)
        thw_sb = wpk_sb[:, 516:646].bitcast(bf16)  # (128, 260)
        wT = wpk_sb[:, 646:774].bitcast(bf16)      # (128, 256)

        b_out = [aux_sb[:, 0:1], aux_sb[:, 1:2]]
        _pgb = aux_sb[:, 4:260]
        pg_bias2 = bass.AP(
            tensor=_pgb.tensor, offset=_pgb.offset,
            ap=[list(_pgb.ap[0]), [0, 2], list(_pgb.ap[1])],
        )

        # m2[d,e] = sum_m g[m,d] p[m,e] accumulates across the whole pg phase
        pm = ps_sm.tile([128, 128], f32, tag="sm")

        # PE warm-up: the HAM clock gate needs ~3.4us of sustained activity
        # to lift the PE from 1.2 to 2.4 GHz; burn the X0-DMA wait on dummy
        # matmuls over the already-resident weights so the real projection
        # runs warm from its first instruction.
        wup = ps_mm.tile([128, 512], f32, tag="mm", name="wup")
        for _ in range(14):
            nc.tensor.matmul(
                wup, lhsT=pgW[:, 0:128], rhs=pgW[:, 0:512],
                start=True, stop=True,
            )

        # pg projection in (N, D)-chunk layout + interleaved m2 accumulation.
        # Each psum tile holds 2 pixel chunks in SEPARATE banks (offsets
        # 0/512) so the k=0 matmuls of many chunks can be pending while X1
        # is still arriving; k=1 + the bias add + m2 matmuls follow.
        pps = []
        for grp in range(NT // 2):          # 12 tiles of 2 chunks
            pp = ps_mm.tile([128, 1024], f32, tag="mm", name=f"pp{grp}")
            pps.append(pp)
            for i in range(2):
                nt = grp * 2 + i
                nsl = slice(nt * 128, (nt + 1) * 128)
                psl = slice(i * 512, i * 512 + 256)
                nc.tensor.matmul(
                    pp[:, psl], lhsT=X0[:, nsl], rhs=pgW[:, 0:256],
                    start=True, stop=False,
                )
        for grp in range(NT // 2):
            pp = pps[grp]
            for i in range(2):
                nt = grp * 2 + i
                nsl = slice(nt * 128, (nt + 1) * 128)
                psl = slice(i * 512, i * 512 + 256)
                nc.tensor.matmul(
                    pp[:, psl], lhsT=X1[:, nsl], rhs=pgW[:, 256:512],
                    start=False, stop=True,
                )
            gsl = slice(grp * 512, (grp + 1) * 512)
            pp_v = bass.AP(
                tensor=pp.tensor, offset=pp.offset,
                ap=[list(pp.ap[0]), [512, 2], [1, 256]],
            )
            nc.vector.tensor_add(
                pg_sb[:, gsl].rearrange("p (a b) -> p a b", a=2),
                pp_v,
                pg_bias2,
            )
            for i in range(2):
                nt = grp * 2 + i
                nc.tensor.matmul(
                    pm,
                    lhsT=pg_sb[:, nt * 256 + 128 : (nt + 1) * 256],
                    rhs=pg_sb[:, nt * 256 : nt * 256 + 128],
                    start=(nt == 0),
                    stop=(nt == NT - 1),
                )
        nc.scalar.copy(out=m2_sb, in_=pm)

        # dummy matmuls keep the PE HAM activity window busy while the small
        # serial m2 -> w2 -> ut -> bz chain runs, so the z matmuls run warm
        def dummy_mms(k):
            dmy = ps_mm.tile([128, 512], f32, tag="mm", name=f"dmy{k}")
            for _ in range(3):
                nc.tensor.matmul(
                    dmy, lhsT=wT[:, 0:128], rhs=pg_sb[:, 0:512],
                    start=True, stop=True,
                )

        dummy_mms(0)

        # w2[e,c] = sum_d m2[d,e] w'[c,d]
        pw = ps_sm.tile([128, 256], f32, tag="sm")
        nc.tensor.matmul(pw, lhsT=m2_sb, rhs=wT, start=True, stop=True)
        nc.scalar.copy(out=w2_sb, in_=pw)
        dummy_mms(1)

        # ut[ci,c] = sum_e theta_w[e,ci] w2[e,c]   (= U[c,ci])
        pu = ps_sm.tile([128, 512], f32, tag="sm2")
        for ci in range(2):
            nc.tensor.matmul(
                pu[:, ci * 256 : (ci + 1) * 256],
                lhsT=thw_sb[:, ci * 128 : (ci + 1) * 128], rhs=w2_sb,
                start=True, stop=True,
            )
        nc.scalar.copy(out=ut_sb, in_=pu)

        # b_z[c] = sum_e w2[e,c] theta_b[e] + b'[c]
        for cc in range(2):
            pb = ps_sm.tile([128, 1], f32, tag="sm")
            nc.tensor.matmul(
                pb, lhsT=w2_sb[:, cc * 128 : (cc + 1) * 128],
                rhs=thw_sb[:, 256:257], start=True, stop=True,
            )
            nc.scalar.activation(
                out=bz_sb[:, cc : cc + 1], in_=pb, func=AF.Identity,
                bias=b_out[cc], scale=1.0,
            )
        dummy_mms(2)

        # z[c,n] = sum_ci U[c,ci] x[ci,n] + b_z[c] + x[c,n]
        ndma = 0
        for j in range(NR):
            for cc in range(2):
                jsl = slice(j * 1024, (j + 1) * 1024)
                pz = ps_mm.tile([128, 1024], f32, tag="mm")
                for f in range(2):
                    fsl = slice(j * 1024 + f * 512, j * 1024 + (f + 1) * 512)
                    psl = slice(f * 512, (f + 1) * 512)
                    nc.tensor.matmul(
                        pz[:, psl],
                        lhsT=ut_sb[:, cc * 128 : (cc + 1) * 128],
                        rhs=X0[:, fsl], start=True, stop=False,
                    )
                    nc.tensor.matmul(
                        pz[:, psl],
                        lhsT=ut_sb[:, 256 + cc * 128 : 256 + (cc + 1) * 128],
                        rhs=X1[:, fsl], start=False, stop=True,
                    )
                xres = (X0 if cc == 0 else X1)[:, jsl]
                z_sb = zpool.tile([128, 1024], f32, tag="z_sb")
                nc.vector.scalar_tensor_tensor(
                    out=z_sb, in0=pz, scalar=bz_sb[:, cc : cc + 1],
                    in1=xres, op0=ALU.add, op1=ALU.add,
                )
                eng = nc.sync if ndma % 2 == 0 else nc.scalar
                ndma += 1
                eng.dma_start(
                    out=out[cc * 128 : (cc + 1) * 128, jsl], in_=z_sb,
                )

    nc.compile()
    return nc


def _get_nc():
    global _NC
    if _NC is None:
        _NC = _build_nc()
    return _NC


# test.py reads this after a traced run to get exec_time_ns
last_results = None


def _prep_inputs(inputs):
    import ml_dtypes

    bf16 = ml_dtypes.bfloat16

    x = np.asarray(inputs["x"], dtype=np.float32)
    theta_w = np.asarray(inputs["theta_w"], np.float32)
    theta_b = np.asarray(inputs["theta_b"], np.float32)
    phi_w = np.asarray(inputs["phi_w"], np.float32)
    phi_b = np.asarray(inputs["phi_b"], np.float32)
    g_w = np.asarray(inputs["g_w"], np.float32)
    g_b = np.asarray(inputs["g_b"], np.float32)
    w_w = np.asarray(inputs["w_w"], np.float32)
    w_b = np.asarray(inputs["w_b"], np.float32)
    bn_gamma = np.asarray(inputs["bn_gamma"], np.float32)
    bn_beta = np.asarray(inputs["bn_beta"], np.float32)
    bn_mean = np.asarray(inputs["bn_mean"], np.float32)
    bn_var = np.asarray(inputs["bn_var"], np.float32)

    inv = bn_gamma / np.sqrt(bn_var + BN_EPS)
    b_out = (w_b - bn_mean) * inv + bn_beta                   # (C,)

    aux = np.zeros((128, 260), np.float32)
    aux[:, 0] = b_out[:128]
    aux[:, 1] = b_out[128:]
    aux[:, 4:260] = np.concatenate([phi_b / N, g_b])[None, :]

    pgw = np.concatenate([phi_w.T / N, g_w.T], axis=1)        # (C, 2D)
    pgw_pk = np.concatenate([pgw[0:128], pgw[128:256]], axis=1)  # (128, 512)
    thw = np.zeros((D, 260), np.float32)
    thw[:, :256] = theta_w
    thw[:, 256] = theta_b
    wwt = (w_w * inv[:, None]).T                              # (D, C)

    wpk_u8 = np.concatenate(
        [
            aux.view(np.uint8),                               # 1040 B
            np.ascontiguousarray(pgw_pk).astype(bf16).view(np.uint8),  # 1024 B
            np.ascontiguousarray(thw).astype(bf16).view(np.uint8),     # 520 B
            np.ascontiguousarray(wwt).astype(bf16).view(np.uint8),     # 512 B
        ],
        axis=1,
    )
    assert wpk_u8.shape == (128, 3096), wpk_u8.shape
    wpk = np.ascontiguousarray(wpk_u8).view(np.float32)       # (128, 774)

    xf = x.reshape(B, C, N).astype(bf16)
    return xf, {"wpk": wpk}


def kernel(**inputs):
    from concourse.bass_utils import run_bass_kernel_spmd

    global last_results

    xf, shared = _prep_inputs(inputs)
    in_maps = [dict(shared, x=np.ascontiguousarray(xf[b])) for b in range(B)]

    nc = _get_nc()
    res = run_bass_kernel_spmd(nc, in_maps, list(range(N_CORES)))
    last_results = res

    z = np.stack([res.results[b]["out"] for b in range(B)])
    return z.reshape(B, C, HH, WW).astype(np.float32)


# revision 16
# speedup vs baseline: 1.0142x; 1.0142x over previous
"""Non-local (dot-product attention) block kernel for Trainium2, 8 cores.

Reference math (per sample):
    t = theta_w @ xf + theta_b           (D, N)
    p = (phi_w @ xf + phi_b) / N         (D, N)
    g = g_w @ xf + g_b                   (D, N)
    f = t.T p  (NxN attention);  y = f g.T;  z = BN(w_w y) + x

Algebraic collapse (matmul associativity, BN folded on host):
    M[e,d] = sum_m p[e,m] g[d,m]                       (D x D)
    V[c,e] = sum_d w'[c,d] M[e,d]       w' = diag(inv) w_w
    z      = V @ t + b' 1^T + x
so the N x N attention matrix and y never exist -- per sample two
projection passes over x (t and [phi|g]) plus a D x D contraction, then
one (C x D) @ (D x N) output matmul.

Sharding: data-parallel over batch B=8, one sample per NeuronCore, no
collectives. Matmul inputs bf16 (fp32 PSUM accumulation), biases and
residual applied in fp32; output fp32.

HW notes baked in:
  - One byte-packed weight DMA (each dma_start has ~600ns fixed cost, and
    in-DMA bandwidth is a shared ~320GB/s pool -> strict need-order:
    weights, X0, X1).
  - Dummy matmuls over a zeroed tile warm the PE HAM clock gate (1.2 ->
    2.4 GHz takes ~3.4us of sustained activity) while x is still in
    flight, and X0-only (k=0) work is emitted ahead of X1-dependent work.
  - theta projection runs interleaved with the phi|g projection (ACT does
    its PSUM->SBUF copies; DVE does the phi|g bias adds), so after the
    tiny M -> V chain the output phase is just 12 K=128 matmuls racing
    the output DMAs.
  - Output DMAs alternate the two HWDGE rings, (128, 1024) fp32 chunks
    (4KB descriptors).
"""

import numpy as np

B, C, HH, WW = 8, 256, 96, 32
N = HH * WW          # 3072
D = 128              # inter_channels
BN_EPS = 1e-5
NT = N // 128        # 24 pixel chunks
NR = N // 1024       # 3 pixel regions
N_CORES = 8

_NC = None


def _build_nc():
    from contextlib import ExitStack

    import concourse.bass as bass
    import concourse.bacc as bacc
    import concourse.tile as tile
    from concourse import mybir

    f32 = mybir.dt.float32
    bf16 = mybir.dt.bfloat16
    AF = mybir.ActivationFunctionType
    ALU = mybir.AluOpType

    nc = bacc.Bacc(
        "TRN2",
        target_bir_lowering=False,
        debug=False,
        num_devices=N_CORES,
    )

    x = nc.dram_tensor("x", [C, N], bf16, kind="ExternalInput").ap()
    # wpk byte-packs, per partition row: aux 260 f32 | pgW 512 bf16 |
    # thw 260 bf16 | wT 256 bf16  => 774 f32 columns total
    wpk = nc.dram_tensor("wpk", [128, 774], f32, kind="ExternalInput").ap()
    out = nc.dram_tensor("out", [C, N], f32, kind="ExternalOutput").ap()

    with tile.TileContext(nc) as tc, ExitStack() as ctx:
        const = ctx.enter_context(tc.tile_pool(name="const", bufs=1))
        zpool = ctx.enter_context(tc.tile_pool(name="zpool", bufs=3))
        ps_mm = ctx.enter_context(tc.tile_pool(name="ps_mm", bufs=3, space="PSUM"))
        ps_sm = ctx.enter_context(tc.tile_pool(name="ps_sm", bufs=1, space="PSUM"))

        X0 = const.tile([128, N], bf16)
        X1 = const.tile([128, N], bf16)
        t_sb = const.tile([128, N], bf16)
        pg_sb = const.tile([128, NT * 256], bf16)
        m2_sb = const.tile([128, 128], bf16)
        w2_sb = const.tile([128, 256], bf16)
        wz = const.tile([128, 512], bf16)
        wpk_sb = const.tile([128, 774], f32)

        nc.sync.dma_start(out=wpk_sb, in_=wpk)
        nc.sync.dma_start(out=X0, in_=x[0:128, :])
        nc.sync.dma_start(out=X1, in_=x[128:256, :])

        aux_sb = wpk_sb[:, 0:260]
        pgW = wpk_sb[:, 260:516].bitcast(bf16)     # (128, 512)
        thw_sb = wpk_sb[:, 516:646].bitcast(bf16)  # (128, 260)
        wT = wpk_sb[:, 646:774].bitcast(bf16)      # (128, 256)

        b_out = [aux_sb[:, 0:1], aux_sb[:, 1:2]]
        theta_b = aux_sb[:, 2:3]
        _pgb = aux_sb[:, 4:260]
        pg_bias4 = bass.AP(
            tensor=_pgb.tensor, offset=_pgb.offset,
            ap=[list(_pgb.ap[0]), [0, 4], list(_pgb.ap[1])],
        )

        # PE warm-up on a zeroed tile: the HAM clock gate needs ~3.4us of
        # sustained activity to lift the PE 1.2 -> 2.4 GHz; burn the x-DMA
        # wait so real matmuls run warm from their first instruction.
        nc.vector.memset(wz, 0.0)
        wup = ps_mm.tile([128, 512], f32, tag="mm", name="wup")
        for _ in range(20):
            nc.tensor.matmul(
                wup, lhsT=wz[:, 0:128], rhs=wz, start=True, stop=True
            )

        # m2[d,e] = sum_m g[m,d] p[m,e] accumulates across the whole pg phase
        pm = ps_sm.tile([128, 128], f32, tag="sm")

        # interleaved theta + phi|g projections.
        #   t group: (128, 512) of t in (D, N), ACT copy w/ theta_b bias
        #   pg group: 4 pixel chunks, DVE bias add via broadcast view,
        #             then 4 m2 accumulation matmuls
        for grp in range(6):
            fsl = slice(grp * 512, (grp + 1) * 512)
            pt = ps_mm.tile([128, 1024], f32, tag="mm", name=f"pt{grp}")
            nc.tensor.matmul(
                pt[:, 0:512], lhsT=thw_sb[:, 0:128], rhs=X0[:, fsl],
                start=True, stop=False,
            )
            nc.tensor.matmul(
                pt[:, 0:512], lhsT=thw_sb[:, 128:256], rhs=X1[:, fsl],
                start=False, stop=True,
            )
            nc.scalar.activation(
                out=t_sb[:, fsl], in_=pt[:, 0:512], func=AF.Identity,
                bias=theta_b, scale=1.0,
            )

            pp = ps_mm.tile([128, 1024], f32, tag="mm", name=f"pp{grp}")
            for i in range(4):
                nt = grp * 4 + i
                nsl = slice(nt * 128, (nt + 1) * 128)
                psl = slice(i * 256, (i + 1) * 256)
                nc.tensor.matmul(
                    pp[:, psl], lhsT=X0[:, nsl], rhs=pgW[:, 0:256],
                    start=True, stop=False,
                )
                nc.tensor.matmul(
                    pp[:, psl], lhsT=X1[:, nsl], rhs=pgW[:, 256:512],
                    start=False, stop=True,
                )
            gsl = slice(grp * 1024, (grp + 1) * 1024)
            nc.vector.tensor_add(
                pg_sb[:, gsl].rearrange("p (a b) -> p a b", a=4),
                pp.rearrange("p (a b) -> p a b", a=4),
                pg_bias4,
            )
            for i in range(4):
                nt = grp * 4 + i
                nc.tensor.matmul(
                    pm,
                    lhsT=pg_sb[:, nt * 256 + 128 : (nt + 1) * 256],
                    rhs=pg_sb[:, nt * 256 : nt * 256 + 128],
                    start=(nt == 0),
                    stop=(nt == NT - 1),
                )
        nc.scalar.copy(out=m2_sb, in_=pm)

        # w2[e,c] = sum_d m2[d,e] w'[c,d]  (= V[c,e])
        pw = ps_sm.tile([128, 256], f32, tag="sm")
        nc.tensor.matmul(pw, lhsT=m2_sb, rhs=wT, start=True, stop=True)
        nc.scalar.copy(out=w2_sb, in_=pw)

        # z[c,n] = sum_e w2[e,c] t[e,n] + b'[c] + x[c,n]
        ndma = 0
        for j in range(NR):
            for cc in range(2):
                jsl = slice(j * 1024, (j + 1) * 1024)
                pz = ps_mm.tile([128, 1024], f32, tag="mm")
                for f in range(2):
                    fsl = slice(j * 1024 + f * 512, j * 1024 + (f + 1) * 512)
                    nc.tensor.matmul(
                        pz[:, f * 512 : (f + 1) * 512],
                        lhsT=w2_sb[:, cc * 128 : (cc + 1) * 128],
                        rhs=t_sb[:, fsl], start=True, stop=True,
                    )
                xres = (X0 if cc == 0 else X1)[:, jsl]
                z_sb = zpool.tile([128, 1024], f32, tag="z_sb")
                nc.vector.scalar_tensor_tensor(
                    out=z_sb, in0=pz, scalar=b_out[cc],
                    in1=xres, op0=ALU.add, op1=ALU.add,
                )
                eng = nc.sync if ndma % 2 == 0 else nc.scalar
                ndma += 1
                eng.dma_start(
                    out=out[cc * 128 : (cc + 1) * 128, jsl], in_=z_sb,
                )

    nc.compile()
    return nc


def _get_nc():
    global _NC
    if _NC is None:
        _NC = _build_nc()
    return _NC


# test.py reads this after a traced run to get exec_time_ns
last_results = None


def _prep_inputs(inputs):
    import ml_dtypes

    bf16 = ml_dtypes.bfloat16

    x = np.asarray(inputs["x"], dtype=np.float32)
    theta_w = np.asarray(inputs["theta_w"], np.float32)
    theta_b = np.asarray(inputs["theta_b"], np.float32)
    phi_w = np.asarray(inputs["phi_w"], np.float32)
    phi_b = np.asarray(inputs["phi_b"], np.float32)
    g_w = np.asarray(inputs["g_w"], np.float32)
    g_b = np.asarray(inputs["g_b"], np.float32)
    w_w = np.asarray(inputs["w_w"], np.float32)
    w_b = np.asarray(inputs["w_b"], np.float32)
    bn_gamma = np.asarray(inputs["bn_gamma"], np.float32)
    bn_beta = np.asarray(inputs["bn_beta"], np.float32)
    bn_mean = np.asarray(inputs["bn_mean"], np.float32)
    bn_var = np.asarray(inputs["bn_var"], np.float32)

    inv = bn_gamma / np.sqrt(bn_var + BN_EPS)
    b_out = (w_b - bn_mean) * inv + bn_beta                   # (C,)

    aux = np.zeros((128, 260), np.float32)
    aux[:, 0] = b_out[:128]
    aux[:, 1] = b_out[128:]
    aux[:, 2] = theta_b
    aux[:, 4:260] = np.concatenate([phi_b / N, g_b])[None, :]

    pgw = np.concatenate([phi_w.T / N, g_w.T], axis=1)        # (C, 2D)
    pgw_pk = np.concatenate([pgw[0:128], pgw[128:256]], axis=1)  # (128, 512)
    # thw holds theta_w^T packed as [c-chunk0 | c-chunk1]: lhsT for the t
    # projection needs (c-part, d-free) = theta_w.T chunks
    thwT = theta_w.T                                          # (C, D)
    thw = np.zeros((D, 260), np.float32)
    thw[:, 0:128] = thwT[0:128, :]
    thw[:, 128:256] = thwT[128:256, :]
    wwt = (w_w * inv[:, None]).T                              # (D, C)

    wpk_u8 = np.concatenate(
        [
            aux.view(np.uint8),                               # 1040 B
            np.ascontiguousarray(pgw_pk).astype(bf16).view(np.uint8),  # 1024 B
            np.ascontiguousarray(thw).astype(bf16).view(np.uint8),     # 520 B
            np.ascontiguousarray(wwt).astype(bf16).view(np.uint8),     # 512 B
        ],
        axis=1,
    )
    assert wpk_u8.shape == (128, 3096), wpk_u8.shape
    wpk = np.ascontiguousarray(wpk_u8).view(np.float32)       # (128, 774)

    xf = x.reshape(B, C, N).astype(bf16)
    return xf, {"wpk": wpk}


def kernel(**inputs):
    from concourse.bass_utils import run_bass_kernel_spmd

    global last_results

    xf, shared = _prep_inputs(inputs)
    in_maps = [dict(shared, x=np.ascontiguousarray(xf[b])) for b in range(B)]

    nc = _get_nc()
    res = run_bass_kernel_spmd(nc, in_maps, list(range(N_CORES)))
    last_results = res

    z = np.stack([res.results[b]["out"] for b in range(B)])
    return z.reshape(B, C, HH, WW).astype(np.float32)
